# revision 1
# baseline (speedup 1.0000x reference)
"""Trainium2 Bass kernel for nn_DiscretizedGaussian (discretized-Gaussian log-likelihood).

Computation per element (mean m, logvar lv, data x):
    idx   = rint(127.5*(x+1))                 (bin index 0..255; int16-cast rint,
                                               verified ties-even on HW)
    cen   = m - idx/128                       (exact -1/128 multiply; int16 read direct)
    iv    = exp(-lv)
    v+-   = (cen + 255/256 +- 1/255) * iv     (CDF eval points of the selected bin)
    z~    = (v^2 + 1/0.044715) * v ;  T = tanh(b2 * z~),  b2 = sqrt(2/pi)*0.044715
    d     = T+ - T-     (Tm computed pre-negated via tanh scale=-b2; the subtract
                         is a DMA-engine accumulate-add, free of compute engines)
    ll    = log(0.5*d + 1e-10)                (== log(max(cdf_diff, 1e-10)), d>=0
                                               as HW tanh is monotone w/ exact sat)
    out_s = sum over all elements of sample s (ACT accum_out + final PE G-matmul).

Engine split per [128, 2048] block (24 blocks/core, 8 cores data-parallel over
batch), from HW-measured per-op costs and the measured fact that GPSIMD fully
serializes with the DVE (shared SBUF port) while ACT/PE/DMA overlap:
    DVE : rint cast, cen (STT), v+/v- (STT), z~ (STT in place over the squares)
    ACT : exp, 2x Square, 2x tanh, ln (+accum_out = free per-partition reduce);
          exp/tanh/ln chained in a grouped order (Ln is in a different ACT
          table set; unordered scheduling thrashes ~2.7us table reloads)
    DMA : inputs on both HWDGE queues (SP + ACT); d-subtract as SWDGE
          accumulate-add SBUF->SBUF
    Pool: only SWDGE descriptor generation for the d-accumulate
    PE  : only the final per-sample G-matmul reduce (fp32 matmul is 1/4 rate on
          TRN2 -- measured 1.08us per [128x128]@[128x512] -- and HAM-cold in
          sparse use, so it is kept off the per-block critical path).
"""
import sys
for _p in ("/opt/trn_rl_repo", "/opt/trn_rl_repo/concourse"):
    if _p not in sys.path:
        sys.path.insert(0, _p)

from contextlib import ExitStack
import numpy as np

import concourse.bass as bass  # noqa: F401
import concourse.tile as tile
from concourse.tile import add_dep_helper
from concourse import bacc, mybir
from concourse import bass_utils

F32 = mybir.dt.float32
I32 = mybir.dt.int32
I16 = mybir.dt.int16
P = 128
FB = 2048                 # free-dim block size
NBLK = 24                 # blocks per core
GRP = 2                   # blocks per ACT-table group
FREE = FB * NBLK          # 49152 free elems per partition per core
NCORE = 8
SPB = 8                   # samples per core (64 / 8)
B, C, H, W = 64, 3, 512, 512

# centered + c0 +- half, where x_sel = idx/128 - 255/256 and half = 1/255
CP = float(np.float64(255.0) / 256.0 + np.float64(1.0) / 255.0)
CM = float(np.float64(255.0) / 256.0 - np.float64(1.0) / 255.0)
CC = float(np.float64(1.0) / np.float64(0.044715))
B2 = float(np.float64(0.7978845608028654) * np.float64(0.044715))

_CACHE = {}


def _consts_np():
    G = np.zeros((P, SPB), np.float32)
    for k in range(P):
        G[k, k // 16] = 1.0
    bias_ln = np.full((P, 1), 1e-10, np.float32)
    return np.ascontiguousarray(
        np.concatenate([G, bias_ln], axis=1), dtype=np.float32)  # [128, 9]


def _build(reps=1):
    A = mybir.AluOpType
    AF = mybir.ActivationFunctionType
    nc = bacc.Bacc(
        "TRN2",
        target_bir_lowering=False,
        debug=False,
        enable_asserts=False,
        num_devices=NCORE,
    )
    m_in = nc.dram_tensor("m_in", [P, FREE], F32, kind="ExternalInput").ap()
    lv_in = nc.dram_tensor("lv_in", [P, FREE], F32, kind="ExternalInput").ap()
    x_in = nc.dram_tensor("x_in", [P, FREE], F32, kind="ExternalInput").ap()
    c_in = nc.dram_tensor("c_in", [P, 9], F32, kind="ExternalInput").ap()
    o_out = nc.dram_tensor("o_out", [1, SPB], F32, kind="ExternalOutput").ap()

    with tile.TileContext(nc) as tc, ExitStack() as ctx:
        pin = ctx.enter_context(tc.tile_pool(name="pin", bufs=2))
        psc = ctx.enter_context(tc.tile_pool(name="psc", bufs=2))
        piv = ctx.enter_context(tc.tile_pool(name="piv", bufs=2))
        pcen = ctx.enter_context(tc.tile_pool(name="pcen", bufs=2))
        pu = ctx.enter_context(tc.tile_pool(name="pu", bufs=4))
        psq = ctx.enter_context(tc.tile_pool(name="psq", bufs=4))
        pTp = ctx.enter_context(tc.tile_pool(name="pTp", bufs=4))
        pTm = ctx.enter_context(tc.tile_pool(name="pTm", bufs=2))
        pone = ctx.enter_context(tc.tile_pool(name="pone", bufs=1))
        pps_o = ctx.enter_context(tc.tile_pool(name="pps_o", bufs=1, space="PSUM"))

        consts = pone.tile([P, 9], F32, tag="consts")
        nc.sync.dma_start(consts[:], c_in[:])
        G = consts[:, 0:8]
        BIAS_LN = consts[:, 8:9]
        partials = pone.tile([P, NBLK], F32, tag="partials")

        act_chain = []

        def act(*args, **kwargs):
            inst = nc.scalar.activation(*args, **kwargs)
            # chain ACT instructions in emission order so the scheduler cannot
            # interleave Ln between Exp/Tanh ops (each interleave costs a
            # ~2.7us ACT table-set reload: exp/tanh vs ln are different sets)
            if act_chain:
                add_dep_helper(inst.ins, act_chain[-1], sync=False,
                               reason="ACT table-set ordering")
            act_chain.append(inst.ins)
            return inst

        def stage1a(b):
            """DMA + rint + exp + cen for block b."""
            c0 = b * FB
            x_t = pin.tile([P, FB], F32, tag="x", name=f"x{b}")
            nc.sync.dma_start(x_t[:], x_in[:, c0:c0 + FB])
            m_t = pin.tile([P, FB], F32, tag="m", name=f"m{b}")
            nc.scalar.dma_start(m_t[:], m_in[:, c0:c0 + FB])
            lv_t = pin.tile([P, FB], F32, tag="lv", name=f"lv{b}")
            nc.sync.dma_start(lv_t[:], lv_in[:, c0:c0 + FB])

            # idx = rint(127.5*(x+1)) via int32-convert (verified ties-even)
            wi_t = psc.tile([P, FB], I16, tag="wi", name=f"wi{b}")
            nc.vector.tensor_scalar(wi_t[:], x_t[:], 1.0, 127.5, A.add, A.mult)

            iv_t = piv.tile([P, FB], F32, tag="iv", name=f"iv{b}")
            act(iv_t[:], lv_t[:], AF.Exp, scale=-1.0)

            # cen = m - idx/128  (int32 wi read directly; -1/128 mult is exact)
            cen_t = pcen.tile([P, FB], F32, tag="cen", name=f"cen{b}")
            nc.vector.scalar_tensor_tensor(cen_t[:], wi_t[:], -0.0078125,
                                           m_t[:], A.mult, A.add)
            return cen_t, iv_t

        def stage1b(b, cen_t, iv_t):
            """u's + squares + z~ + tanh + d for block b."""
            up_t = pu.tile([P, FB], F32, tag="u", name=f"up{b}")
            um_t = pu.tile([P, FB], F32, tag="u", name=f"um{b}")
            nc.vector.scalar_tensor_tensor(up_t[:], cen_t[:], CP,
                                           iv_t[:], A.add, A.mult)
            nc.vector.scalar_tensor_tensor(um_t[:], cen_t[:], CM,
                                           iv_t[:], A.add, A.mult)

            sp_t = psq.tile([P, FB], F32, tag="s", name=f"sp{b}")
            # unchained: Square is in every relevant ACT table set, so its
            # position never causes a table reload -- let the scheduler float it
            nc.scalar.activation(sp_t[:], up_t[:], AF.Square)
            sm_t = psq.tile([P, FB], F32, tag="s", name=f"sm{b}")
            nc.scalar.activation(sm_t[:], um_t[:], AF.Square)

            # z~ = (s + CC) * u, in place over s
            nc.vector.scalar_tensor_tensor(sp_t[:], sp_t[:], CC, up_t[:],
                                           A.add, A.mult)
            nc.vector.scalar_tensor_tensor(sm_t[:], sm_t[:], CC, um_t[:],
                                           A.add, A.mult)

            Tp_t = pTp.tile([P, FB], F32, tag="Tp", name=f"Tp{b}")
            act(Tp_t[:], sp_t[:], AF.Tanh, scale=B2)
            Tm_t = pTm.tile([P, FB], F32, tag="Tm", name=f"Tm{b}")
            act(Tm_t[:], sm_t[:], AF.Tanh, scale=-B2)   # = -tanh(B2 z~m)
            # d = T+ - T- accumulated in place over Tp by the DMA engines
            nc.gpsimd.dma_start(Tp_t[:], Tm_t[:], accum_op=A.add)
            return Tp_t

        def stage2(b, d_t):
            """Deferred ln+accum (ACT) for block b; input d held in the Tp tile."""
            act(d_t[:], d_t[:], AF.Ln,
                bias=BIAS_LN, scale=0.5,
                accum_out=partials[:, b:b + 1])

        def full_pass(_i=None):
            # ACT chain order per group: [exp x GRP] [deferred ln of group g-1]
            # [tanh x 2*GRP] -- 2 table-set switches per group, and exp lands
            # early so DVE's u-ops are never starved of iv.
            pend = []
            for g in range(NBLK // GRP):
                blocks = [g * GRP + i for i in range(GRP)]
                s1 = [stage1a(b) for b in blocks]
                for b, d_t in pend:
                    stage2(b, d_t)
                ds = [stage1b(b, *s1[i]) for i, b in enumerate(blocks)]
                pend = [(blocks[i], ds[i]) for i in range(GRP)]
            for b, d_t in pend:
                stage2(b, d_t)

        if reps == 1:
            full_pass()
        else:
            tc.For_i_unrolled(0, reps, 1, full_pass, max_unroll=1)

        part_sum = pone.tile([P, 1], F32, tag="psum1")
        nc.vector.tensor_reduce(part_sum[:], partials[:],
                                axis=mybir.AxisListType.X, op=A.add)
        out_ps = pps_o.tile([1, SPB], F32, tag="outp", name="outp")
        nc.tensor.matmul(out_ps[:], part_sum[:], G, start=True, stop=True)
        out_sb = pone.tile([1, SPB], F32, tag="outs")
        nc.vector.tensor_copy(out_sb[:], out_ps[:])
        nc.sync.dma_start(o_out[:], out_sb[:])
    nc.compile()
    return nc


def _get_nc(reps=1):
    key = f"nc{reps}"
    if key not in _CACHE:
        _CACHE[key] = _build(reps)
    return _CACHE[key]


def _make_in_maps(mean, logvar, x):
    consts = _consts_np()
    in_maps = []
    for k in range(NCORE):
        sl = slice(k * SPB, (k + 1) * SPB)
        in_maps.append({
            "m_in": np.ascontiguousarray(mean[sl], dtype=np.float32).reshape(P, FREE),
            "lv_in": np.ascontiguousarray(logvar[sl], dtype=np.float32).reshape(P, FREE),
            "x_in": np.ascontiguousarray(x[sl], dtype=np.float32).reshape(P, FREE),
            "c_in": consts,
        })
    return in_maps


def _run(in_maps, trace=False):
    nc = _get_nc()
    return bass_utils.run_bass_kernel_spmd(
        nc, in_maps, core_ids=list(range(NCORE)), trace=trace)


def kernel(mean, logvar, x):
    assert mean.shape == (B, C, H, W), mean.shape
    res = _run(_make_in_maps(mean, logvar, x), trace=False)
    out = np.concatenate([r["o_out"].reshape(SPB) for r in res.results])
    return out.astype(np.float32)


if __name__ == "__main__":
    rng = np.random.default_rng(0)
    m = (rng.standard_normal((B, C, H, W)) * 0.1).astype(np.float32)
    lv = (rng.standard_normal((B, C, H, W)) * 0.1 - 2.0).astype(np.float32)
    xx = rng.uniform(-1.0, 1.0 - 1e-6, (B, C, H, W)).astype(np.float32)
    out = kernel(m, lv, xx)
    print("kernel out[:8]:", out[:8])



# revision 2
# speedup vs baseline: 1.3043x; 1.3043x over previous
"""Trainium2 Bass kernel for nn_DiscretizedGaussian (discretized-Gaussian log-likelihood).

Computation per element (mean m, logvar lv, data x):
    idx   = rint(127.5*(x+1))                 (bin index 0..255; int16-cast rint)
    cen   = m - idx/128                       (exact -1/128 multiply)
    iv    = exp(-lv)
    u+-   = (cen + 255/256 +- 1/255) * iv     (CDF eval points of the selected bin)
    z+-   = (u^2 + 1/0.044715) * u
    T+    = tanh(B2*z+), Tm = tanh(-B2*z-)    (B2 = sqrt(2/pi)*0.044715; Tm pre-negated)
    d     = T+ + Tm      (DMA-engine accumulate-add, free of compute engines)
    ll    = log(0.5*d + 1e-10)
    out_s = sum over all elements of sample s (ACT accum_out + final PE G-matmul).

v2 changes vs the fp32 baseline (465.7us/pass):
  - inputs are downcast to BF16 on the host: halves HBM DMA traffic
    (210us -> 105us/core) and unlocks DVE 2x/4x perf modes.
  - all DVE stages run in bf16 (cast 4x, STT/TT 2x_1P): DVE drops
    ~305us -> ~200us.  tanh outputs stay FP32 (the d = Tp - Tm
    cancellation needs full precision); ln reads fp32.
  - squares moved from ACT to DVE (TT bf16 2x); ACT does only
    exp, 2x tanh, ln = 4 elem-passes (~7.4us/block vs 13.8us).
  - branch pair (up|um) packed in one [128, 2*FB] tile so square and
    z-build are single wide instructions (halves DVE instruction count).
  - per-sample reduction epilogue unchanged (ACT accum_out + PE G-matmul).
Engine budget per [128,2048] block: DVE ~8.0us, ACT ~7.4us + table-switch
amortization, DMA ~4.6us in + ~3us d-accum.  24 blocks/core, 8 cores
data-parallel over batch.
"""
import sys
for _p in ("/opt/trn_rl_repo", "/opt/trn_rl_repo/concourse"):
    if _p not in sys.path:
        sys.path.insert(0, _p)

from contextlib import ExitStack
import numpy as np
import ml_dtypes

import concourse.bass as bass  # noqa: F401
import concourse.tile as tile
from concourse.tile import add_dep_helper
from concourse import bacc, mybir
from concourse import bass_utils

F32 = mybir.dt.float32
BF16 = mybir.dt.bfloat16
I16 = mybir.dt.int16
NPBF = ml_dtypes.bfloat16
P = 128
FB = 2048                 # free-dim block size
NBLK = 24                 # blocks per core
GRP = 4                   # blocks per ACT-table group
FREE = FB * NBLK          # 49152 free elems per partition per core
NCORE = 8
SPB = 8                   # samples per core (64 / 8)
B, C, H, W = 64, 3, 512, 512

# u = (cen + c0 +- half) * iv, where x_sel = idx/128 - 255/256, half = 1/255
CP = float(np.float64(255.0) / 256.0 + np.float64(1.0) / 255.0)
CM = float(np.float64(255.0) / 256.0 - np.float64(1.0) / 255.0)
CC = float(np.float64(1.0) / np.float64(0.044715))
B2 = float(np.float64(0.7978845608028654) * np.float64(0.044715))

# mid-stage dtype: BF16 (fast) or F32 (safe fallback if rel err too big)
MID = BF16

_CACHE = {}


def _consts_np():
    G = np.zeros((P, SPB), np.float32)
    for k in range(P):
        G[k, k // 16] = 1.0
    bias_ln = np.full((P, 1), 1e-10, np.float32)
    return np.ascontiguousarray(
        np.concatenate([G, bias_ln], axis=1), dtype=np.float32)  # [128, 9]


def _build(reps=1):
    A = mybir.AluOpType
    AF = mybir.ActivationFunctionType
    nc = bacc.Bacc(
        "TRN2",
        target_bir_lowering=False,
        debug=False,
        enable_asserts=False,
        num_devices=NCORE,
    )
    m_in = nc.dram_tensor("m_in", [P, FREE], BF16, kind="ExternalInput").ap()
    lv_in = nc.dram_tensor("lv_in", [P, FREE], BF16, kind="ExternalInput").ap()
    x_in = nc.dram_tensor("x_in", [P, FREE], BF16, kind="ExternalInput").ap()
    c_in = nc.dram_tensor("c_in", [P, 9], F32, kind="ExternalInput").ap()
    o_out = nc.dram_tensor("o_out", [1, SPB], F32, kind="ExternalOutput").ap()

    with tile.TileContext(nc) as tc, ExitStack() as ctx:
        pin = ctx.enter_context(tc.tile_pool(name="pin", bufs=2))
        psc = ctx.enter_context(tc.tile_pool(name="psc", bufs=2))
        piv = ctx.enter_context(tc.tile_pool(name="piv", bufs=2))
        pcen = ctx.enter_context(tc.tile_pool(name="pcen", bufs=2))
        pu = ctx.enter_context(tc.tile_pool(name="pu", bufs=2))
        psq = ctx.enter_context(tc.tile_pool(name="psq", bufs=2))
        pTp = ctx.enter_context(tc.tile_pool(name="pTp", bufs=GRP + 1))
        pTm = ctx.enter_context(tc.tile_pool(name="pTm", bufs=2))
        pone = ctx.enter_context(tc.tile_pool(name="pone", bufs=1))
        pps_o = ctx.enter_context(tc.tile_pool(name="pps_o", bufs=1, space="PSUM"))

        consts = pone.tile([P, 9], F32, tag="consts")
        nc.sync.dma_start(consts[:], c_in[:])
        G = consts[:, 0:8]
        BIAS_LN = consts[:, 8:9]
        partials = pone.tile([P, NBLK], F32, tag="partials")

        act_chain = []

        def act(*args, **kwargs):
            inst = nc.scalar.activation(*args, **kwargs)
            # chain ACT instructions in emission order so the scheduler cannot
            # interleave Ln between Exp/Tanh ops (each interleave costs a
            # ~2.7us ACT table-set reload: exp/tanh vs ln are different sets)
            if act_chain:
                add_dep_helper(inst.ins, act_chain[-1], sync=False,
                               reason="ACT table-set ordering")
            act_chain.append(inst.ins)
            return inst

        def stage1a(b):
            """DMA + rint + exp + cen for block b."""
            c0 = b * FB
            x_t = pin.tile([P, FB], BF16, tag="x", name=f"x{b}")
            nc.sync.dma_start(x_t[:], x_in[:, c0:c0 + FB])
            m_t = pin.tile([P, FB], BF16, tag="m", name=f"m{b}")
            nc.scalar.dma_start(m_t[:], m_in[:, c0:c0 + FB])
            lv_t = pin.tile([P, FB], BF16, tag="lv", name=f"lv{b}")
            nc.sync.dma_start(lv_t[:], lv_in[:, c0:c0 + FB])

            # idx = rint(127.5*(x+1)) via int16-convert (ties-even)
            wi_t = psc.tile([P, FB], I16, tag="wi", name=f"wi{b}")
            nc.vector.tensor_scalar(wi_t[:], x_t[:], 1.0, 127.5, A.add, A.mult)

            iv_t = piv.tile([P, FB], MID, tag="iv", name=f"iv{b}")
            act(iv_t[:], lv_t[:], AF.Exp, scale=-1.0)

            # cen = m - idx/128  (int16 wi read directly; -1/128 mult is exact)
            cen_t = pcen.tile([P, FB], MID, tag="cen", name=f"cen{b}")
            nc.vector.scalar_tensor_tensor(cen_t[:], wi_t[:], -0.0078125,
                                           m_t[:], A.mult, A.add)
            return cen_t, iv_t

        def stage1b(b, cen_t, iv_t):
            """u's + squares + z + tanh + d for block b."""
            u_t = pu.tile([P, 2 * FB], MID, tag="u", name=f"u{b}")
            nc.vector.scalar_tensor_tensor(u_t[:, 0:FB], cen_t[:], CP,
                                           iv_t[:], A.add, A.mult)
            nc.vector.scalar_tensor_tensor(u_t[:, FB:2 * FB], cen_t[:], CM,
                                           iv_t[:], A.add, A.mult)

            # s = u*u (packed pair, one wide TT), then z = (s + CC)*u in place
            s_t = psq.tile([P, 2 * FB], MID, tag="s", name=f"s{b}")
            nc.vector.tensor_tensor(s_t[:], u_t[:], u_t[:], A.mult)
            nc.vector.scalar_tensor_tensor(s_t[:], s_t[:], CC, u_t[:],
                                           A.add, A.mult)

            Tp_t = pTp.tile([P, FB], F32, tag="Tp", name=f"Tp{b}")
            act(Tp_t[:], s_t[:, 0:FB], AF.Tanh, scale=B2)
            Tm_t = pTm.tile([P, FB], F32, tag="Tm", name=f"Tm{b}")
            act(Tm_t[:], s_t[:, FB:2 * FB], AF.Tanh, scale=-B2)  # = -tanh(B2 z-)
            # d = T+ - T- accumulated in place over Tp by the DMA engines
            nc.gpsimd.dma_start(Tp_t[:], Tm_t[:], accum_op=A.add)
            return Tp_t

        def stage2(b, d_t):
            """Deferred ln+accum (ACT) for block b; input d held in the Tp tile."""
            act(d_t[:], d_t[:], AF.Ln,
                bias=BIAS_LN, scale=0.5,
                accum_out=partials[:, b:b + 1])

        def full_pass(_i=None):
            # ACT chain order per group: [exp x GRP] [deferred ln of group g-1]
            # [tanh x 2*GRP] -- 2 table-set switches per group, and exp lands
            # early so DVE's u-ops are never starved of iv.
            pend = []
            for g in range(NBLK // GRP):
                blocks = [g * GRP + i for i in range(GRP)]
                s1 = [stage1a(b) for b in blocks]
                for b, d_t in pend:
                    stage2(b, d_t)
                ds = [stage1b(b, *s1[i]) for i, b in enumerate(blocks)]
                pend = [(blocks[i], ds[i]) for i in range(GRP)]
            for b, d_t in pend:
                stage2(b, d_t)

        if reps == 1:
            full_pass()
        else:
            tc.For_i_unrolled(0, reps, 1, full_pass, max_unroll=1)

        part_sum = pone.tile([P, 1], F32, tag="psum1")
        nc.vector.tensor_reduce(part_sum[:], partials[:],
                                axis=mybir.AxisListType.X, op=A.add)
        out_ps = pps_o.tile([1, SPB], F32, tag="outp", name="outp")
        nc.tensor.matmul(out_ps[:], part_sum[:], G, start=True, stop=True)
        out_sb = pone.tile([1, SPB], F32, tag="outs")
        nc.vector.tensor_copy(out_sb[:], out_ps[:])
        nc.sync.dma_start(o_out[:], out_sb[:])
    nc.compile()
    return nc


def _get_nc(reps=1):
    key = f"nc{reps}"
    if key not in _CACHE:
        _CACHE[key] = _build(reps)
    return _CACHE[key]


def _make_in_maps(mean, logvar, x):
    consts = _consts_np()
    mb = np.ascontiguousarray(mean, dtype=np.float32).reshape(NCORE * P, FREE).astype(NPBF)
    lvb = np.ascontiguousarray(logvar, dtype=np.float32).reshape(NCORE * P, FREE).astype(NPBF)
    xb = np.ascontiguousarray(x, dtype=np.float32).reshape(NCORE * P, FREE).astype(NPBF)
    in_maps = []
    for k in range(NCORE):
        sl = slice(k * P, (k + 1) * P)
        in_maps.append({
            "m_in": mb[sl],
            "lv_in": lvb[sl],
            "x_in": xb[sl],
            "c_in": consts,
        })
    return in_maps


# ---- persistent-jit fast path (avoids per-call retrace/XLA recompile) ----

def _prep_fast(nc):
    import jax
    from jax.sharding import Mesh, PartitionSpec, NamedSharding
    from jax.experimental.shard_map import shard_map
    from concourse.bass2jax import (_bass_exec_p, install_neuronx_cc_hook,
                                    partition_id_tensor)
    install_neuronx_cc_hook()
    partition_name = nc.partition_id_tensor.name if nc.partition_id_tensor else None
    in_names, out_names, out_avals, zero_shapes = [], [], [], []
    for alloc in nc.m.functions[0].allocations:
        if not isinstance(alloc, mybir.MemoryLocationSet):
            continue
        name = alloc.memorylocations[0].name
        if alloc.kind == "ExternalInput":
            if name != partition_name:
                in_names.append(name)
        elif alloc.kind == "ExternalOutput":
            out_names.append(name)
            shape = tuple(alloc.tensor_shape)
            dtype = mybir.dt.np(alloc.dtype)
            out_avals.append(jax.core.ShapedArray(shape, dtype))
            zero_shapes.append((shape, dtype))
    n_params = len(in_names)
    n_outs = len(out_avals)
    all_names = list(in_names) + list(out_names)
    if partition_name is not None:
        all_names.append(partition_name)
    donate = tuple(range(n_params, n_params + n_outs))

    def _body(*args):
        operands = list(args)
        if partition_name is not None:
            operands.append(partition_id_tensor())
        outs = _bass_exec_p.bind(
            *operands,
            out_avals=tuple(out_avals),
            in_names=tuple(all_names),
            out_names=tuple(out_names),
            lowering_input_output_aliases=(),
            sim_require_finite=True,
            sim_require_nnan=True,
            nc=nc,
        )
        return tuple(outs)

    devices = jax.devices()[:NCORE]
    mesh = Mesh(np.asarray(devices), ("core",))
    fn = jax.jit(
        shard_map(_body, mesh=mesh,
                  in_specs=(PartitionSpec("core"),) * (n_params + n_outs),
                  out_specs=(PartitionSpec("core"),) * n_outs,
                  check_rep=False),
        donate_argnums=donate, keep_unused=True,
    )
    sharding = NamedSharding(mesh, PartitionSpec("core"))
    return fn, in_names, out_names, out_avals, zero_shapes, sharding


def _get_fast(nc):
    if "fast" not in _CACHE:
        _CACHE["fast"] = _prep_fast(nc)
    return _CACHE["fast"]


def kernel(mean, logvar, x):
    import jax
    assert mean.shape == (B, C, H, W), mean.shape
    nc = _get_nc()
    fn, in_names, out_names, out_avals, zero_shapes, sharding = _get_fast(nc)
    full = {
        "m_in": np.asarray(mean, dtype=np.float32).reshape(NCORE * P, FREE).astype(NPBF),
        "lv_in": np.asarray(logvar, dtype=np.float32).reshape(NCORE * P, FREE).astype(NPBF),
        "x_in": np.asarray(x, dtype=np.float32).reshape(NCORE * P, FREE).astype(NPBF),
        "c_in": np.tile(_consts_np(), (NCORE, 1)),
    }
    args = [jax.device_put(full[nm], sharding) for nm in in_names]
    zeros = [jax.device_put(np.zeros((NCORE * s[0], *s[1:]), d), sharding)
             for (s, d) in zero_shapes]
    outs = fn(*args, *zeros)
    oi = out_names.index("o_out")
    res = np.asarray(outs[oi]).reshape(NCORE, SPB)
    return res.reshape(B).astype(np.float32)


if __name__ == "__main__":
    rng = np.random.default_rng(0)
    m = (rng.standard_normal((B, C, H, W)) * 0.1).astype(np.float32)
    lv = (rng.standard_normal((B, C, H, W)) * 0.1 - 2.0).astype(np.float32)
    xx = rng.uniform(-1.0, 1.0 - 1e-6, (B, C, H, W)).astype(np.float32)
    out = kernel(m, lv, xx)
    print("kernel out[:8]:", out[:8])


# revision 11
# speedup vs baseline: 1.4615x; 1.1205x over previous
"""Trainium2 Bass kernel for nn_DiscretizedGaussian (discretized-Gaussian log-likelihood).

Computation per element (mean m, logvar lv, data x):
    idx   = rint(127.5*(x+1))                 (bin index 0..255; int16-cast rint)
    cen   = m - idx/128                       (exact -1/128 multiply)
    iv    = exp(-lv)
    u+-   = (cen + 255/256 +- 1/255) * iv     (CDF eval points of the selected bin)
    z+-   = (u^2 + 1/0.044715) * u
    T+    = tanh(B2*z+), Tm = tanh(-B2*z-)    (B2 = sqrt(2/pi)*0.044715; Tm pre-negated)
    d     = T+ + Tm      (DMA-engine accumulate-add, free of compute engines)
    ll    = log(0.5*d + 1e-10)
    out_s = sum over all elements of sample s (ACT accum_out + final PE G-matmul).

v2 changes vs the fp32 baseline (465.7us/pass):
  - inputs are downcast to BF16 on the host: halves HBM DMA traffic
    (210us -> 105us/core) and unlocks DVE 2x/4x perf modes.
  - all DVE stages run in bf16 (cast 4x, STT/TT 2x_1P): DVE drops
    ~305us -> ~200us.  tanh outputs stay FP32 (the d = Tp - Tm
    cancellation needs full precision); ln reads fp32.
  - squares moved from ACT to DVE (TT bf16 2x); ACT does only
    exp, 2x tanh, ln = 4 elem-passes (~7.4us/block vs 13.8us).
  - branch pair (up|um) packed in one [128, 2*FB] tile so square and
    z-build are single wide instructions (halves DVE instruction count).
  - per-sample reduction epilogue unchanged (ACT accum_out + PE G-matmul).
Engine budget per [128,2048] block: DVE ~8.0us, ACT ~7.4us + table-switch
amortization, DMA ~4.6us in + ~3us d-accum.  24 blocks/core, 8 cores
data-parallel over batch.
"""
import sys
for _p in ("/opt/trn_rl_repo", "/opt/trn_rl_repo/concourse"):
    if _p not in sys.path:
        sys.path.insert(0, _p)

from contextlib import ExitStack
import numpy as np
import ml_dtypes

import concourse.bass as bass  # noqa: F401
import concourse.tile as tile
from concourse.tile import add_dep_helper
from concourse import bacc, mybir
from concourse import bass_utils

F32 = mybir.dt.float32
BF16 = mybir.dt.bfloat16
I16 = mybir.dt.int16
NPBF = ml_dtypes.bfloat16
P = 128
FB = 2048                 # free-dim block size
NBLK = 24                 # blocks per core
GRP = 4                   # blocks per ACT-table group
DEFER = 2                 # groups of ln deferral (hide d-accum DMA latency)
FREE = FB * NBLK          # 49152 free elems per partition per core
NCORE = 8
SPB = 8                   # samples per core (64 / 8)
B, C, H, W = 64, 3, 512, 512

# u = (cen + c0 +- half) * iv, where x_sel = idx/128 - 255/256, half = 1/255
CP = float(np.float64(255.0) / 256.0 + np.float64(1.0) / 255.0)
CM = float(np.float64(255.0) / 256.0 - np.float64(1.0) / 255.0)
CC = float(np.float64(1.0) / np.float64(0.044715))
B2 = float(np.float64(0.7978845608028654) * np.float64(0.044715))

# mid-stage dtype: BF16 (fast) or F32 (safe fallback if rel err too big)
MID = BF16

_CACHE = {}


def _consts_np():
    G = np.zeros((P, SPB), np.float32)
    for k in range(P):
        G[k, k // 16] = 1.0
    bias_ln = np.full((P, 1), 1e-10, np.float32)
    dhalf = np.full((P, 1), np.float64(255.0) / 256.0 + np.float64(1.0) / 255.0
                    - (np.float64(255.0) / 256.0 - np.float64(1.0) / 255.0),
                    np.float32)  # CP - CM = 2/255
    return np.ascontiguousarray(
        np.concatenate([G, bias_ln, dhalf], axis=1), dtype=np.float32)  # [128, 10]


def _build(reps=1):
    A = mybir.AluOpType
    AF = mybir.ActivationFunctionType
    nc = bacc.Bacc(
        "TRN2",
        target_bir_lowering=False,
        debug=False,
        enable_asserts=False,
        num_devices=NCORE,
    )
    m_in = nc.dram_tensor("m_in", [P, FREE], BF16, kind="ExternalInput").ap()
    lv_in = nc.dram_tensor("lv_in", [P, FREE], BF16, kind="ExternalInput").ap()
    x_in = nc.dram_tensor("x_in", [P, FREE], BF16, kind="ExternalInput").ap()
    c_in = nc.dram_tensor("c_in", [P, 10], F32, kind="ExternalInput").ap()
    o_out = nc.dram_tensor("o_out", [1, SPB], F32, kind="ExternalOutput").ap()

    with tile.TileContext(nc) as tc, ExitStack() as ctx:
        pin = ctx.enter_context(tc.tile_pool(name="pin", bufs=3))
        psc = ctx.enter_context(tc.tile_pool(name="psc", bufs=3))
        piv = ctx.enter_context(tc.tile_pool(name="piv", bufs=3))
        pu = ctx.enter_context(tc.tile_pool(name="pu", bufs=2))
        psq = ctx.enter_context(tc.tile_pool(name="psq", bufs=2))
        pTp = ctx.enter_context(tc.tile_pool(name="pTp", bufs=DEFER * GRP + 1))
        pTm = ctx.enter_context(tc.tile_pool(name="pTm", bufs=2))
        pone = ctx.enter_context(tc.tile_pool(name="pone", bufs=1))
        pps_o = ctx.enter_context(tc.tile_pool(name="pps_o", bufs=1, space="PSUM"))

        consts = pone.tile([P, 10], F32, tag="consts")
        nc.sync.dma_start(consts[:], c_in[:])
        G = consts[:, 0:8]
        BIAS_LN = consts[:, 8:9]
        BIAS_DHALF = consts[:, 9:10]
        partials = pone.tile([P, NBLK], F32, tag="partials")

        act_chain = []

        def act(*args, **kwargs):
            inst = nc.scalar.activation(*args, **kwargs)
            # chain ACT instructions in emission order so the scheduler cannot
            # interleave Ln between Exp/Tanh ops (each interleave costs a
            # ~2.7us ACT table-set reload: exp/tanh vs ln are different sets)
            if act_chain:
                add_dep_helper(inst.ins, act_chain[-1], sync=False,
                               reason="ACT table-set ordering")
            act_chain.append(inst.ins)
            return inst

        def stage1a(b):
            """DMA + rint + exp + cen for block b."""
            c0 = b * FB
            x_t = pin.tile([P, FB], BF16, tag="x", name=f"x{b}")
            nc.sync.dma_start(x_t[:], x_in[:, c0:c0 + FB])
            m_t = pin.tile([P, FB], BF16, tag="m", name=f"m{b}")
            nc.sync.dma_start(m_t[:], m_in[:, c0:c0 + FB])
            lv_t = pin.tile([P, FB], BF16, tag="lv", name=f"lv{b}")
            nc.sync.dma_start(lv_t[:], lv_in[:, c0:c0 + FB])

            # idx = rint(127.5*(x+1)) via int16-convert (ties-even); TS is 4x
            wi_t = psc.tile([P, FB], I16, tag="wi", name=f"wi{b}")
            nc.vector.tensor_scalar(wi_t[:], x_t[:], 1.0, 127.5, A.add, A.mult)

            iv_t = piv.tile([P, FB], MID, tag="iv", name=f"iv{b}")
            act(iv_t[:], lv_t[:], AF.Exp, scale=-1.0)

            return wi_t, m_t, iv_t

        def stage1b(b, wi_t, m_t, iv_t):
            """u's + squares + z + tanh + d for block b (all TS/TT, no STT --
            STT runs at 1x always, no perf-mode uops).

            u+pre = (wi*(-1/128) + CP) + m  == cen + CP   (TS folds CP)
            u-pre = -u+pre + (CP-CM)       == -(cen + CM) (pre-negated)
            so both tanhs use scale=+B2 and tanh of the minus half lands
            pre-negated as the d-accum needs.
            """
            u_t = pu.tile([P, 2 * FB], MID, tag="u", name=f"u{b}")
            nc.vector.tensor_scalar(u_t[:, 0:FB], wi_t[:], -0.0078125, CP,
                                    A.mult, A.add)
            nc.vector.tensor_tensor(u_t[:, 0:FB], u_t[:, 0:FB], m_t[:], A.add)
            nc.vector.tensor_scalar(u_t[:, FB:2 * FB], u_t[:, 0:FB], -1.0,
                                    CP - CM, A.mult, A.add)
            nc.vector.tensor_tensor(u_t[:, 0:FB], u_t[:, 0:FB], iv_t[:], A.mult)
            nc.vector.tensor_tensor(u_t[:, FB:2 * FB], u_t[:, FB:2 * FB],
                                    iv_t[:], A.mult)

            # s = u*u (one wide TT over the packed pair), z = (s+CC)*u:
            # wide TS (+CC in place), then wide TT mult (in place over s)
            s_t = psq.tile([P, 2 * FB], MID, tag="s", name=f"s{b}")
            nc.vector.tensor_tensor(s_t[:], u_t[:], u_t[:], A.mult)
            nc.vector.tensor_scalar(s_t[:], s_t[:], CC, None, A.add)
            nc.vector.tensor_tensor(s_t[:], s_t[:], u_t[:], A.mult)

            Tp_t = pTp.tile([P, FB], F32, tag="Tp", name=f"Tp{b}")
            act(Tp_t[:], s_t[:, 0:FB], AF.Tanh, scale=B2)
            Tm_t = pTm.tile([P, FB], F32, tag="Tm", name=f"Tm{b}")
            act(Tm_t[:], s_t[:, FB:2 * FB], AF.Tanh, scale=B2)  # z- pre-negated
            # d = T+ - T- accumulated in place over Tp by the DMA engines
            nc.gpsimd.dma_start(Tp_t[:], Tm_t[:], accum_op=A.add)
            return Tp_t

        def stage2(b, d_t):
            """Deferred ln+accum (ACT) for block b; input d held in the Tp tile."""
            act(d_t[:], d_t[:], AF.Ln,
                bias=BIAS_LN, scale=0.5,
                accum_out=partials[:, b:b + 1])

        def full_pass(_i=None):
            # ACT chain order per group: [exp x GRP] [deferred ln of group g-1]
            # [tanh x 2*GRP] -- 2 table-set switches per group, and exp lands
            # early so DVE's u-ops are never starved of iv.
            pend = []
            for g in range(NBLK // GRP):
                blocks = [g * GRP + i for i in range(GRP)]
                s1 = [stage1a(b) for b in blocks]
                if len(pend) >= DEFER:
                    for b, d_t in pend.pop(0):
                        stage2(b, d_t)
                ds = [stage1b(b, *s1[i]) for i, b in enumerate(blocks)]
                pend.append([(blocks[i], ds[i]) for i in range(GRP)])
            for grp_pend in pend:
                for b, d_t in grp_pend:
                    stage2(b, d_t)

        if reps == 1:
            full_pass()
        else:
            tc.For_i_unrolled(0, reps, 1, full_pass, max_unroll=1)

        part_sum = pone.tile([P, 1], F32, tag="psum1")
        nc.vector.tensor_reduce(part_sum[:], partials[:],
                                axis=mybir.AxisListType.X, op=A.add)
        out_ps = pps_o.tile([1, SPB], F32, tag="outp", name="outp")
        nc.tensor.matmul(out_ps[:], part_sum[:], G, start=True, stop=True)
        out_sb = pone.tile([1, SPB], F32, tag="outs")
        nc.vector.tensor_copy(out_sb[:], out_ps[:])
        nc.sync.dma_start(o_out[:], out_sb[:])
    nc.compile()
    return nc


def _get_nc(reps=1):
    key = f"nc{reps}"
    if key not in _CACHE:
        _CACHE[key] = _build(reps)
    return _CACHE[key]


def _make_in_maps(mean, logvar, x):
    consts = _consts_np()
    mb = np.ascontiguousarray(mean, dtype=np.float32).reshape(NCORE * P, FREE).astype(NPBF)
    lvb = np.ascontiguousarray(logvar, dtype=np.float32).reshape(NCORE * P, FREE).astype(NPBF)
    xb = np.ascontiguousarray(x, dtype=np.float32).reshape(NCORE * P, FREE).astype(NPBF)
    in_maps = []
    for k in range(NCORE):
        sl = slice(k * P, (k + 1) * P)
        in_maps.append({
            "m_in": mb[sl],
            "lv_in": lvb[sl],
            "x_in": xb[sl],
            "c_in": consts,
        })
    return in_maps


# ---- persistent-jit fast path (avoids per-call retrace/XLA recompile) ----

def _prep_fast(nc):
    import jax
    from jax.sharding import Mesh, PartitionSpec, NamedSharding
    from jax.experimental.shard_map import shard_map
    from concourse.bass2jax import (_bass_exec_p, install_neuronx_cc_hook,
                                    partition_id_tensor)
    install_neuronx_cc_hook()
    partition_name = nc.partition_id_tensor.name if nc.partition_id_tensor else None
    in_names, out_names, out_avals, zero_shapes = [], [], [], []
    for alloc in nc.m.functions[0].allocations:
        if not isinstance(alloc, mybir.MemoryLocationSet):
            continue
        name = alloc.memorylocations[0].name
        if alloc.kind == "ExternalInput":
            if name != partition_name:
                in_names.append(name)
        elif alloc.kind == "ExternalOutput":
            out_names.append(name)
            shape = tuple(alloc.tensor_shape)
            dtype = mybir.dt.np(alloc.dtype)
            out_avals.append(jax.core.ShapedArray(shape, dtype))
            zero_shapes.append((shape, dtype))
    n_params = len(in_names)
    n_outs = len(out_avals)
    all_names = list(in_names) + list(out_names)
    if partition_name is not None:
        all_names.append(partition_name)
    donate = tuple(range(n_params, n_params + n_outs))

    def _body(*args):
        operands = list(args)
        if partition_name is not None:
            operands.append(partition_id_tensor())
        outs = _bass_exec_p.bind(
            *operands,
            out_avals=tuple(out_avals),
            in_names=tuple(all_names),
            out_names=tuple(out_names),
            lowering_input_output_aliases=(),
            sim_require_finite=True,
            sim_require_nnan=True,
            nc=nc,
        )
        return tuple(outs)

    devices = jax.devices()[:NCORE]
    mesh = Mesh(np.asarray(devices), ("core",))
    fn = jax.jit(
        shard_map(_body, mesh=mesh,
                  in_specs=(PartitionSpec("core"),) * (n_params + n_outs),
                  out_specs=(PartitionSpec("core"),) * n_outs,
                  check_rep=False),
        donate_argnums=donate, keep_unused=True,
    )
    sharding = NamedSharding(mesh, PartitionSpec("core"))
    return fn, in_names, out_names, out_avals, zero_shapes, sharding


def _get_fast(nc):
    if "fast" not in _CACHE:
        _CACHE["fast"] = _prep_fast(nc)
    return _CACHE["fast"]


def kernel(mean, logvar, x):
    import jax
    assert mean.shape == (B, C, H, W), mean.shape
    nc = _get_nc()
    fn, in_names, out_names, out_avals, zero_shapes, sharding = _get_fast(nc)
    full = {
        "m_in": np.asarray(mean, dtype=np.float32).reshape(NCORE * P, FREE).astype(NPBF),
        "lv_in": np.asarray(logvar, dtype=np.float32).reshape(NCORE * P, FREE).astype(NPBF),
        "x_in": np.asarray(x, dtype=np.float32).reshape(NCORE * P, FREE).astype(NPBF),
        "c_in": np.tile(_consts_np(), (NCORE, 1)),
    }
    args = [jax.device_put(full[nm], sharding) for nm in in_names]
    zeros = [jax.device_put(np.zeros((NCORE * s[0], *s[1:]), d), sharding)
             for (s, d) in zero_shapes]
    outs = fn(*args, *zeros)
    oi = out_names.index("o_out")
    res = np.asarray(outs[oi]).reshape(NCORE, SPB)
    return res.reshape(B).astype(np.float32)


if __name__ == "__main__":
    rng = np.random.default_rng(0)
    m = (rng.standard_normal((B, C, H, W)) * 0.1).astype(np.float32)
    lv = (rng.standard_normal((B, C, H, W)) * 0.1 - 2.0).astype(np.float32)
    xx = rng.uniform(-1.0, 1.0 - 1e-6, (B, C, H, W)).astype(np.float32)
    out = kernel(m, lv, xx)
    print("kernel out[:8]:", out[:8])


# revision 13
# speedup vs baseline: 1.4688x; 1.0050x over previous
"""Trainium2 Bass kernel for nn_DiscretizedGaussian (discretized-Gaussian log-likelihood).

Computation per element (mean m, logvar lv, data x):
    idx   = rint(127.5*(x+1))                 (bin index 0..255; int16-cast rint)
    cen   = m - idx/128                       (exact -1/128 multiply)
    iv    = exp(-lv)
    u+-   = (cen + 255/256 +- 1/255) * iv     (CDF eval points of the selected bin)
    z+-   = (u^2 + 1/0.044715) * u
    T+    = tanh(B2*z+), Tm = tanh(-B2*z-)    (B2 = sqrt(2/pi)*0.044715; Tm pre-negated)
    d     = T+ + Tm      (DMA-engine accumulate-add, free of compute engines)
    ll    = log(0.5*d + 1e-10)
    out_s = sum over all elements of sample s (ACT accum_out + final PE G-matmul).

v2 changes vs the fp32 baseline (465.7us/pass):
  - inputs are downcast to BF16 on the host: halves HBM DMA traffic
    (210us -> 105us/core) and unlocks DVE 2x/4x perf modes.
  - all DVE stages run in bf16 (cast 4x, STT/TT 2x_1P): DVE drops
    ~305us -> ~200us.  tanh outputs stay FP32 (the d = Tp - Tm
    cancellation needs full precision); ln reads fp32.
  - squares moved from ACT to DVE (TT bf16 2x); ACT does only
    exp, 2x tanh, ln = 4 elem-passes (~7.4us/block vs 13.8us).
  - branch pair (up|um) packed in one [128, 2*FB] tile so square and
    z-build are single wide instructions (halves DVE instruction count).
  - per-sample reduction epilogue unchanged (ACT accum_out + PE G-matmul).
Engine budget per [128,2048] block: DVE ~8.0us, ACT ~7.4us + table-switch
amortization, DMA ~4.6us in + ~3us d-accum.  24 blocks/core, 8 cores
data-parallel over batch.
"""
import sys
for _p in ("/opt/trn_rl_repo", "/opt/trn_rl_repo/concourse"):
    if _p not in sys.path:
        sys.path.insert(0, _p)

from contextlib import ExitStack
import numpy as np
import ml_dtypes

import concourse.bass as bass  # noqa: F401
import concourse.tile as tile
from concourse.tile import add_dep_helper
from concourse import bacc, mybir
from concourse import bass_utils

F32 = mybir.dt.float32
BF16 = mybir.dt.bfloat16
I16 = mybir.dt.int16
NPBF = ml_dtypes.bfloat16
P = 128
FB = 2048                 # free-dim block size
NBLK = 24                 # blocks per core
GRP = 4                   # blocks per ACT-table group
DEFER = 2                 # groups of ln deferral (hide d-accum DMA latency)
FREE = FB * NBLK          # 49152 free elems per partition per core
NCORE = 8
SPB = 8                   # samples per core (64 / 8)
B, C, H, W = 64, 3, 512, 512

# u = (cen + c0 +- half) * iv, where x_sel = idx/128 - 255/256, half = 1/255
CP = float(np.float64(255.0) / 256.0 + np.float64(1.0) / 255.0)
CM = float(np.float64(255.0) / 256.0 - np.float64(1.0) / 255.0)
CC = float(np.float64(1.0) / np.float64(0.044715))
B2 = float(np.float64(0.7978845608028654) * np.float64(0.044715))

# mid-stage dtype: BF16 (fast) or F32 (safe fallback if rel err too big)
MID = BF16

_CACHE = {}


def _consts_np():
    G = np.zeros((P, SPB), np.float32)
    for k in range(P):
        G[k, k // 16] = 1.0
    bias_ln = np.full((P, 1), 1e-10, np.float32)
    dhalf = np.full((P, 1), np.float64(255.0) / 256.0 + np.float64(1.0) / 255.0
                    - (np.float64(255.0) / 256.0 - np.float64(1.0) / 255.0),
                    np.float32)  # CP - CM = 2/255
    return np.ascontiguousarray(
        np.concatenate([G, bias_ln, dhalf], axis=1), dtype=np.float32)  # [128, 10]


def _build(reps=1):
    A = mybir.AluOpType
    AF = mybir.ActivationFunctionType
    nc = bacc.Bacc(
        "TRN2",
        target_bir_lowering=False,
        debug=False,
        enable_asserts=False,
        num_devices=NCORE,
    )
    m_in = nc.dram_tensor("m_in", [P, FREE], BF16, kind="ExternalInput").ap()
    lv_in = nc.dram_tensor("lv_in", [P, FREE], BF16, kind="ExternalInput").ap()
    x_in = nc.dram_tensor("x_in", [P, FREE], BF16, kind="ExternalInput").ap()
    c_in = nc.dram_tensor("c_in", [P, 10], F32, kind="ExternalInput").ap()
    o_out = nc.dram_tensor("o_out", [1, SPB], F32, kind="ExternalOutput").ap()

    with tile.TileContext(nc) as tc, ExitStack() as ctx:
        pin = ctx.enter_context(tc.tile_pool(name="pin", bufs=3))
        psc = ctx.enter_context(tc.tile_pool(name="psc", bufs=3))
        piv = ctx.enter_context(tc.tile_pool(name="piv", bufs=3))
        pu = ctx.enter_context(tc.tile_pool(name="pu", bufs=2))
        psq = ctx.enter_context(tc.tile_pool(name="psq", bufs=2))
        pTp = ctx.enter_context(tc.tile_pool(name="pTp", bufs=DEFER * GRP + 1))
        pTm = ctx.enter_context(tc.tile_pool(name="pTm", bufs=2))
        pone = ctx.enter_context(tc.tile_pool(name="pone", bufs=1))
        pps_o = ctx.enter_context(tc.tile_pool(name="pps_o", bufs=1, space="PSUM"))

        consts = pone.tile([P, 10], F32, tag="consts")
        nc.sync.dma_start(consts[:], c_in[:])
        G = consts[:, 0:8]
        BIAS_LN = consts[:, 8:9]
        BIAS_DHALF = consts[:, 9:10]
        partials = pone.tile([P, NBLK], F32, tag="partials")

        act_chain = []

        def act(*args, **kwargs):
            inst = nc.scalar.activation(*args, **kwargs)
            # chain ACT instructions in emission order so the scheduler cannot
            # interleave Ln between Exp/Tanh ops (each interleave costs a
            # ~2.7us ACT table-set reload: exp/tanh vs ln are different sets)
            if act_chain:
                add_dep_helper(inst.ins, act_chain[-1], sync=False,
                               reason="ACT table-set ordering")
            act_chain.append(inst.ins)
            return inst

        def stage1a(b):
            """DMA + rint + exp + cen for block b."""
            c0 = b * FB
            x_t = pin.tile([P, FB], BF16, tag="x", name=f"x{b}")
            nc.sync.dma_start(x_t[:], x_in[:, c0:c0 + FB])
            m_t = pin.tile([P, FB], BF16, tag="m", name=f"m{b}")
            nc.sync.dma_start(m_t[:], m_in[:, c0:c0 + FB])
            lv_t = pin.tile([P, FB], BF16, tag="lv", name=f"lv{b}")
            nc.sync.dma_start(lv_t[:], lv_in[:, c0:c0 + FB])

            # idx = rint(127.5*(x+1)) via int16-convert (ties-even); TS is 4x
            wi_t = psc.tile([P, FB], I16, tag="wi", name=f"wi{b}")
            nc.vector.tensor_scalar(wi_t[:], x_t[:], 1.0, 127.5, A.add, A.mult)

            iv_t = piv.tile([P, FB], MID, tag="iv", name=f"iv{b}")
            act(iv_t[:], lv_t[:], AF.Exp, scale=-1.0)

            return wi_t, m_t, iv_t

        def stage1b(b, wi_t, m_t, iv_t):
            """u's + squares + z + tanh + d for block b (all TS/TT, no STT --
            STT runs at 1x always, no perf-mode uops).

            u+pre = (wi*(-1/128) + CP) + m  == cen + CP   (TS folds CP)
            u-pre = -u+pre + (CP-CM)       == -(cen + CM) (pre-negated)
            so both tanhs use scale=+B2 and tanh of the minus half lands
            pre-negated as the d-accum needs.
            """
            u_t = pu.tile([P, 2 * FB], MID, tag="u", name=f"u{b}")
            nc.vector.tensor_scalar(u_t[:, 0:FB], wi_t[:], -0.0078125, CP,
                                    A.mult, A.add)
            nc.vector.tensor_tensor(u_t[:, 0:FB], u_t[:, 0:FB], m_t[:], A.add)
            nc.vector.tensor_scalar(u_t[:, FB:2 * FB], u_t[:, 0:FB], -1.0,
                                    CP - CM, A.mult, A.add)
            nc.vector.tensor_tensor(u_t[:, 0:FB], u_t[:, 0:FB], iv_t[:], A.mult)
            nc.vector.tensor_tensor(u_t[:, FB:2 * FB], u_t[:, FB:2 * FB],
                                    iv_t[:], A.mult)

            # s = u*u (one wide TT over the packed pair), z = (s+CC)*u:
            # wide TS (+CC in place), then wide TT mult (in place over s)
            s_t = psq.tile([P, 2 * FB], MID, tag="s", name=f"s{b}")
            nc.vector.tensor_tensor(s_t[:], u_t[:], u_t[:], A.mult)
            nc.vector.tensor_scalar(s_t[:], s_t[:], CC, None, A.add)
            nc.vector.tensor_tensor(s_t[:], s_t[:], u_t[:], A.mult)

            Tp_t = pTp.tile([P, FB], F32, tag="Tp", name=f"Tp{b}")
            act(Tp_t[:], s_t[:, 0:FB], AF.Tanh, scale=B2)
            Tm_t = pTm.tile([P, FB], F32, tag="Tm", name=f"Tm{b}")
            act(Tm_t[:], s_t[:, FB:2 * FB], AF.Tanh, scale=B2)  # z- pre-negated
            # d = T+ - T- accumulated in place over Tp by the DMA engines
            nc.gpsimd.dma_start(Tp_t[:], Tm_t[:], accum_op=A.add)
            return Tp_t

        def stage2(b, d_t):
            """Deferred ln+accum (ACT) for block b; input d held in the Tp tile."""
            act(d_t[:], d_t[:], AF.Ln,
                bias=BIAS_LN, scale=0.5,
                accum_out=partials[:, b:b + 1])

        def full_pass(_i=None):
            # ACT chain order per group: [exp x GRP] [deferred ln of group g-1]
            # [tanh x 2*GRP] -- 2 table-set switches per group, and exp lands
            # early so DVE's u-ops are never starved of iv.
            pend = []
            for g in range(NBLK // GRP):
                blocks = [g * GRP + i for i in range(GRP)]
                s1 = [stage1a(b) for b in blocks]
                if len(pend) >= DEFER:
                    for b, d_t in pend.pop(0):
                        stage2(b, d_t)
                ds = [stage1b(b, *s1[i]) for i, b in enumerate(blocks)]
                pend.append([(blocks[i], ds[i]) for i in range(GRP)])
            for grp_pend in pend:
                for b, d_t in grp_pend:
                    stage2(b, d_t)

        if reps == 1:
            full_pass()
        else:
            tc.For_i_unrolled(0, reps, 1, full_pass, max_unroll=1)

        part_sum = pone.tile([P, 1], F32, tag="psum1")
        nc.vector.tensor_reduce(part_sum[:], partials[:],
                                axis=mybir.AxisListType.X, op=A.add)
        out_ps = pps_o.tile([1, SPB], F32, tag="outp", name="outp")
        nc.tensor.matmul(out_ps[:], part_sum[:], G, start=True, stop=True)
        out_sb = pone.tile([1, SPB], F32, tag="outs")
        nc.vector.tensor_copy(out_sb[:], out_ps[:])
        nc.sync.dma_start(o_out[:], out_sb[:])
    nc.compile()
    return nc


def _get_nc(reps=1):
    key = f"nc{reps}"
    if key not in _CACHE:
        _CACHE[key] = _build(reps)
    return _CACHE[key]


def _make_in_maps(mean, logvar, x):
    consts = _consts_np()
    mb = np.ascontiguousarray(mean, dtype=np.float32).reshape(NCORE * P, FREE).astype(NPBF)
    lvb = np.ascontiguousarray(logvar, dtype=np.float32).reshape(NCORE * P, FREE).astype(NPBF)
    xb = np.ascontiguousarray(x, dtype=np.float32).reshape(NCORE * P, FREE).astype(NPBF)
    in_maps = []
    for k in range(NCORE):
        sl = slice(k * P, (k + 1) * P)
        in_maps.append({
            "m_in": mb[sl],
            "lv_in": lvb[sl],
            "x_in": xb[sl],
            "c_in": consts,
        })
    return in_maps


# ---- persistent-jit fast path (avoids per-call retrace/XLA recompile) ----

def _prep_fast(nc):
    import jax
    from jax.sharding import Mesh, PartitionSpec, NamedSharding
    from jax.experimental.shard_map import shard_map
    from concourse.bass2jax import (_bass_exec_p, install_neuronx_cc_hook,
                                    partition_id_tensor)
    install_neuronx_cc_hook()
    partition_name = nc.partition_id_tensor.name if nc.partition_id_tensor else None
    in_names, out_names, out_avals, zero_shapes = [], [], [], []
    for alloc in nc.m.functions[0].allocations:
        if not isinstance(alloc, mybir.MemoryLocationSet):
            continue
        name = alloc.memorylocations[0].name
        if alloc.kind == "ExternalInput":
            if name != partition_name:
                in_names.append(name)
        elif alloc.kind == "ExternalOutput":
            out_names.append(name)
            shape = tuple(alloc.tensor_shape)
            dtype = mybir.dt.np(alloc.dtype)
            out_avals.append(jax.core.ShapedArray(shape, dtype))
            zero_shapes.append((shape, dtype))
    n_params = len(in_names)
    n_outs = len(out_avals)
    all_names = list(in_names) + list(out_names)
    if partition_name is not None:
        all_names.append(partition_name)
    donate = tuple(range(n_params, n_params + n_outs))

    def _body(*args):
        operands = list(args)
        if partition_name is not None:
            operands.append(partition_id_tensor())
        outs = _bass_exec_p.bind(
            *operands,
            out_avals=tuple(out_avals),
            in_names=tuple(all_names),
            out_names=tuple(out_names),
            lowering_input_output_aliases=(),
            sim_require_finite=True,
            sim_require_nnan=True,
            nc=nc,
        )
        return tuple(outs)

    devices = jax.devices()[:NCORE]
    mesh = Mesh(np.asarray(devices), ("core",))
    fn = jax.jit(
        shard_map(_body, mesh=mesh,
                  in_specs=(PartitionSpec("core"),) * (n_params + n_outs),
                  out_specs=(PartitionSpec("core"),) * n_outs,
                  check_rep=False),
        donate_argnums=donate, keep_unused=True,
    )
    sharding = NamedSharding(mesh, PartitionSpec("core"))
    return fn, in_names, out_names, out_avals, zero_shapes, sharding


def _get_fast(nc):
    if "fast" not in _CACHE:
        _CACHE["fast"] = _prep_fast(nc)
    return _CACHE["fast"]


def kernel(mean, logvar, x):
    import jax
    assert mean.shape == (B, C, H, W), mean.shape
    nc = _get_nc()
    fn, in_names, out_names, out_avals, zero_shapes, sharding = _get_fast(nc)
    full = {
        "m_in": np.asarray(mean, dtype=np.float32).reshape(NCORE * P, FREE).astype(NPBF),
        "lv_in": np.asarray(logvar, dtype=np.float32).reshape(NCORE * P, FREE).astype(NPBF),
        "x_in": np.asarray(x, dtype=np.float32).reshape(NCORE * P, FREE).astype(NPBF),
        "c_in": np.tile(_consts_np(), (NCORE, 1)),
    }
    args = [jax.device_put(full[nm], sharding) for nm in in_names]
    zeros = [jax.device_put(np.zeros((NCORE * s[0], *s[1:]), d), sharding)
             for (s, d) in zero_shapes]
    outs = fn(*args, *zeros)
    oi = out_names.index("o_out")
    res = np.asarray(outs[oi]).reshape(NCORE, SPB)
    return res.reshape(B).astype(np.float32)


if __name__ == "__main__":
    rng = np.random.default_rng(0)
    m = (rng.standard_normal((B, C, H, W)) * 0.1).astype(np.float32)
    lv = (rng.standard_normal((B, C, H, W)) * 0.1 - 2.0).astype(np.float32)
    xx = rng.uniform(-1.0, 1.0 - 1e-6, (B, C, H, W)).astype(np.float32)
    out = kernel(m, lv, xx)
    print("kernel out[:8]:", out[:8])


# revision 15
# speedup vs baseline: 1.4745x; 1.0039x over previous
"""Trainium2 Bass kernel for nn_DiscretizedGaussian (discretized-Gaussian log-likelihood).

Computation per element (mean m, logvar lv, data x), all math identical to the
fp32 reference formula (the tanh-approximated normal CDF):
    idx   = rint(127.5*(x+1))              (bin index 0..255, int16-convert rint)
    u+pre = (-idx/128 + CP) + m            (== cen + c0 + half;  CP folded into
                                            the idx-scaling TS so `cen` is never
                                            materialized)
    u-pre = -u+pre + (CP-CM)               (== -(cen + c0 - half), pre-negated)
    u+-   = u+-pre * exp(-lv)
    z+-   = (u^2 + 1/0.044715) * u         (z- lands pre-negated via u-)
    T+-   = tanh(B2 * z+-)  fp32           (B2 = sqrt(2/pi)*0.044715; T- is
                                            -tanh(B2*z-) automatically)
    d     = T+ + T-                        (DMA-engine accumulate-add, free of
                                            compute engines; d >= 0 by tanh
                                            monotonicity)
    ll    = log(0.5*d + 1e-10)             (ACT Ln, accum_out per-partition sum)
    out_s = per-sample total via PE G-matmul epilogue.

Performance design (465.7us fp32 baseline -> 317us measured):
  - Inputs downcast to BF16 on the host: halves HBM DMA (210->105us/core) and
    unlocks DVE perf modes.  Measured end-to-end rel err 2.3e-3 (gate 2e-2);
    T/d/ln stay fp32 because d = T+ - T- cancels catastrophically below fp32.
  - DVE uses ONLY tensor_scalar (4x for 16-bit dtypes) and tensor_tensor
    (2x_1p bf16): scalar_tensor_tensor has no perf-mode uops and always runs
    1x (cost-model verified), so every STT was rewritten as TS+TT.
  - Squares/cubics on DVE as wide [128,2*FB] packed-pair TTs over [u+|u-];
    ACT does only exp, 2x tanh, ln (4 passes/block ~7.7us) vs DVE ~10.6us.
  - ACT chain order per group of GRP=4 blocks: [exp x4][ln of group g-2 x4]
    [tanh x8] -> 2 table-set loads per group (exp/tanh share a set; ln is
    foreign).  Ln is deferred TWO groups (DEFER=2) so it never waits on the
    d-accumulate DMA of the group just issued - this was worth ~60us.
  - Engine busy (CoreSim, matches HW within ~10%): DVE 256us (bottleneck),
    ACT 199us, SP-DGE 114us, Pool 76us; total sim 288us, HW 317us.
  24 blocks/core of [128, 2048]; 8 cores data-parallel over batch; host-side
  persistent jit + sharded device_put keeps warm kernel() calls ~3.4s
  (dominated by the 302MB axon transfer).
"""
import sys
for _p in ("/opt/trn_rl_repo", "/opt/trn_rl_repo/concourse"):
    if _p not in sys.path:
        sys.path.insert(0, _p)

from contextlib import ExitStack
import numpy as np
import ml_dtypes

import concourse.bass as bass  # noqa: F401
import concourse.tile as tile
from concourse.tile import add_dep_helper
from concourse import bacc, mybir
from concourse import bass_utils

F32 = mybir.dt.float32
BF16 = mybir.dt.bfloat16
I16 = mybir.dt.int16
NPBF = ml_dtypes.bfloat16
P = 128
FB = 2048                 # free-dim block size
NBLK = 24                 # blocks per core
GRP = 4                   # blocks per ACT-table group
DEFER = 2                 # groups of ln deferral (hide d-accum DMA latency)
FREE = FB * NBLK          # 49152 free elems per partition per core
NCORE = 8
SPB = 8                   # samples per core (64 / 8)
B, C, H, W = 64, 3, 512, 512

# u = (cen + c0 +- half) * iv, where x_sel = idx/128 - 255/256, half = 1/255
CP = float(np.float64(255.0) / 256.0 + np.float64(1.0) / 255.0)
CM = float(np.float64(255.0) / 256.0 - np.float64(1.0) / 255.0)
CC = float(np.float64(1.0) / np.float64(0.044715))
B2 = float(np.float64(0.7978845608028654) * np.float64(0.044715))

# mid-stage dtype: BF16 (fast) or F32 (safe fallback if rel err too big)
MID = BF16

_CACHE = {}


def _consts_np():
    G = np.zeros((P, SPB), np.float32)
    for k in range(P):
        G[k, k // 16] = 1.0
    bias_ln = np.full((P, 1), 1e-10, np.float32)
    dhalf = np.full((P, 1), np.float64(255.0) / 256.0 + np.float64(1.0) / 255.0
                    - (np.float64(255.0) / 256.0 - np.float64(1.0) / 255.0),
                    np.float32)  # CP - CM = 2/255
    return np.ascontiguousarray(
        np.concatenate([G, bias_ln, dhalf], axis=1), dtype=np.float32)  # [128, 10]


def _build(reps=1):
    A = mybir.AluOpType
    AF = mybir.ActivationFunctionType
    nc = bacc.Bacc(
        "TRN2",
        target_bir_lowering=False,
        debug=False,
        enable_asserts=False,
        num_devices=NCORE,
    )
    m_in = nc.dram_tensor("m_in", [P, FREE], BF16, kind="ExternalInput").ap()
    lv_in = nc.dram_tensor("lv_in", [P, FREE], BF16, kind="ExternalInput").ap()
    x_in = nc.dram_tensor("x_in", [P, FREE], BF16, kind="ExternalInput").ap()
    c_in = nc.dram_tensor("c_in", [P, 10], F32, kind="ExternalInput").ap()
    o_out = nc.dram_tensor("o_out", [1, SPB], F32, kind="ExternalOutput").ap()

    with tile.TileContext(nc) as tc, ExitStack() as ctx:
        pin = ctx.enter_context(tc.tile_pool(name="pin", bufs=3))
        psc = ctx.enter_context(tc.tile_pool(name="psc", bufs=3))
        piv = ctx.enter_context(tc.tile_pool(name="piv", bufs=3))
        pu = ctx.enter_context(tc.tile_pool(name="pu", bufs=2))
        psq = ctx.enter_context(tc.tile_pool(name="psq", bufs=2))
        pTp = ctx.enter_context(tc.tile_pool(name="pTp", bufs=DEFER * GRP + 1))
        pTm = ctx.enter_context(tc.tile_pool(name="pTm", bufs=2))
        pone = ctx.enter_context(tc.tile_pool(name="pone", bufs=1))
        pps_o = ctx.enter_context(tc.tile_pool(name="pps_o", bufs=1, space="PSUM"))

        consts = pone.tile([P, 10], F32, tag="consts")
        nc.sync.dma_start(consts[:], c_in[:])
        G = consts[:, 0:8]
        BIAS_LN = consts[:, 8:9]
        BIAS_DHALF = consts[:, 9:10]
        partials = pone.tile([P, NBLK], F32, tag="partials")

        act_chain = []

        def act(*args, **kwargs):
            inst = nc.scalar.activation(*args, **kwargs)
            # chain ACT instructions in emission order so the scheduler cannot
            # interleave Ln between Exp/Tanh ops (each interleave costs a
            # ~2.7us ACT table-set reload: exp/tanh vs ln are different sets)
            if act_chain:
                add_dep_helper(inst.ins, act_chain[-1], sync=False,
                               reason="ACT table-set ordering")
            act_chain.append(inst.ins)
            return inst

        def stage1a(b):
            """DMA + rint + exp + cen for block b."""
            c0 = b * FB
            x_t = pin.tile([P, FB], BF16, tag="x", name=f"x{b}")
            nc.sync.dma_start(x_t[:], x_in[:, c0:c0 + FB])
            m_t = pin.tile([P, FB], BF16, tag="m", name=f"m{b}")
            nc.sync.dma_start(m_t[:], m_in[:, c0:c0 + FB])
            lv_t = pin.tile([P, FB], BF16, tag="lv", name=f"lv{b}")
            nc.sync.dma_start(lv_t[:], lv_in[:, c0:c0 + FB])

            # idx = rint(127.5*(x+1)) via int16-convert (ties-even); TS is 4x
            wi_t = psc.tile([P, FB], I16, tag="wi", name=f"wi{b}")
            nc.vector.tensor_scalar(wi_t[:], x_t[:], 1.0, 127.5, A.add, A.mult)

            iv_t = piv.tile([P, FB], MID, tag="iv", name=f"iv{b}")
            act(iv_t[:], lv_t[:], AF.Exp, scale=-1.0)

            return wi_t, m_t, iv_t

        def stage1b(b, wi_t, m_t, iv_t):
            """u's + squares + z + tanh + d for block b (all TS/TT, no STT --
            STT runs at 1x always, no perf-mode uops).

            u+pre = (wi*(-1/128) + CP) + m  == cen + CP   (TS folds CP)
            u-pre = -u+pre + (CP-CM)       == -(cen + CM) (pre-negated)
            so both tanhs use scale=+B2 and tanh of the minus half lands
            pre-negated as the d-accum needs.
            """
            u_t = pu.tile([P, 2 * FB], MID, tag="u", name=f"u{b}")
            nc.vector.tensor_scalar(u_t[:, 0:FB], wi_t[:], -0.0078125, CP,
                                    A.mult, A.add)
            nc.vector.tensor_tensor(u_t[:, 0:FB], u_t[:, 0:FB], m_t[:], A.add)
            nc.vector.tensor_scalar(u_t[:, FB:2 * FB], u_t[:, 0:FB], -1.0,
                                    CP - CM, A.mult, A.add)
            nc.vector.tensor_tensor(u_t[:, 0:FB], u_t[:, 0:FB], iv_t[:], A.mult)
            nc.vector.tensor_tensor(u_t[:, FB:2 * FB], u_t[:, FB:2 * FB],
                                    iv_t[:], A.mult)

            # s = u*u (one wide TT over the packed pair), z = (s+CC)*u:
            # wide TS (+CC in place), then wide TT mult (in place over s)
            s_t = psq.tile([P, 2 * FB], MID, tag="s", name=f"s{b}")
            nc.vector.tensor_tensor(s_t[:], u_t[:], u_t[:], A.mult)
            nc.vector.tensor_scalar(s_t[:], s_t[:], CC, None, A.add)
            nc.vector.tensor_tensor(s_t[:], s_t[:], u_t[:], A.mult)

            Tp_t = pTp.tile([P, FB], F32, tag="Tp", name=f"Tp{b}")
            act(Tp_t[:], s_t[:, 0:FB], AF.Tanh, scale=B2)
            Tm_t = pTm.tile([P, FB], F32, tag="Tm", name=f"Tm{b}")
            act(Tm_t[:], s_t[:, FB:2 * FB], AF.Tanh, scale=B2)  # z- pre-negated
            # d = T+ - T- accumulated in place over Tp by the DMA engines
            nc.gpsimd.dma_start(Tp_t[:], Tm_t[:], accum_op=A.add)
            return Tp_t

        def stage2(b, d_t):
            """Deferred ln+accum (ACT) for block b; input d held in the Tp tile."""
            act(d_t[:], d_t[:], AF.Ln,
                bias=BIAS_LN, scale=0.5,
                accum_out=partials[:, b:b + 1])

        def full_pass(_i=None):
            # ACT chain order per group: [exp x GRP] [deferred ln of group g-1]
            # [tanh x 2*GRP] -- 2 table-set switches per group, and exp lands
            # early so DVE's u-ops are never starved of iv.
            pend = []
            for g in range(NBLK // GRP):
                blocks = [g * GRP + i for i in range(GRP)]
                s1 = [stage1a(b) for b in blocks]
                if len(pend) >= DEFER:
                    for b, d_t in pend.pop(0):
                        stage2(b, d_t)
                ds = [stage1b(b, *s1[i]) for i, b in enumerate(blocks)]
                pend.append([(blocks[i], ds[i]) for i in range(GRP)])
            for grp_pend in pend:
                for b, d_t in grp_pend:
                    stage2(b, d_t)

        if reps == 1:
            full_pass()
        else:
            tc.For_i_unrolled(0, reps, 1, full_pass, max_unroll=1)

        part_sum = pone.tile([P, 1], F32, tag="psum1")
        nc.vector.tensor_reduce(part_sum[:], partials[:],
                                axis=mybir.AxisListType.X, op=A.add)
        out_ps = pps_o.tile([1, SPB], F32, tag="outp", name="outp")
        nc.tensor.matmul(out_ps[:], part_sum[:], G, start=True, stop=True)
        out_sb = pone.tile([1, SPB], F32, tag="outs")
        nc.vector.tensor_copy(out_sb[:], out_ps[:])
        nc.sync.dma_start(o_out[:], out_sb[:])
    nc.compile()
    return nc


def _get_nc(reps=1):
    key = f"nc{reps}"
    if key not in _CACHE:
        _CACHE[key] = _build(reps)
    return _CACHE[key]


def _make_in_maps(mean, logvar, x):
    consts = _consts_np()
    mb = np.ascontiguousarray(mean, dtype=np.float32).reshape(NCORE * P, FREE).astype(NPBF)
    lvb = np.ascontiguousarray(logvar, dtype=np.float32).reshape(NCORE * P, FREE).astype(NPBF)
    xb = np.ascontiguousarray(x, dtype=np.float32).reshape(NCORE * P, FREE).astype(NPBF)
    in_maps = []
    for k in range(NCORE):
        sl = slice(k * P, (k + 1) * P)
        in_maps.append({
            "m_in": mb[sl],
            "lv_in": lvb[sl],
            "x_in": xb[sl],
            "c_in": consts,
        })
    return in_maps


# ---- persistent-jit fast path (avoids per-call retrace/XLA recompile) ----

def _prep_fast(nc):
    import jax
    from jax.sharding import Mesh, PartitionSpec, NamedSharding
    from jax.experimental.shard_map import shard_map
    from concourse.bass2jax import (_bass_exec_p, install_neuronx_cc_hook,
                                    partition_id_tensor)
    install_neuronx_cc_hook()
    partition_name = nc.partition_id_tensor.name if nc.partition_id_tensor else None
    in_names, out_names, out_avals, zero_shapes = [], [], [], []
    for alloc in nc.m.functions[0].allocations:
        if not isinstance(alloc, mybir.MemoryLocationSet):
            continue
        name = alloc.memorylocations[0].name
        if alloc.kind == "ExternalInput":
            if name != partition_name:
                in_names.append(name)
        elif alloc.kind == "ExternalOutput":
            out_names.append(name)
            shape = tuple(alloc.tensor_shape)
            dtype = mybir.dt.np(alloc.dtype)
            out_avals.append(jax.core.ShapedArray(shape, dtype))
            zero_shapes.append((shape, dtype))
    n_params = len(in_names)
    n_outs = len(out_avals)
    all_names = list(in_names) + list(out_names)
    if partition_name is not None:
        all_names.append(partition_name)
    donate = tuple(range(n_params, n_params + n_outs))

    def _body(*args):
        operands = list(args)
        if partition_name is not None:
            operands.append(partition_id_tensor())
        outs = _bass_exec_p.bind(
            *operands,
            out_avals=tuple(out_avals),
            in_names=tuple(all_names),
            out_names=tuple(out_names),
            lowering_input_output_aliases=(),
            sim_require_finite=True,
            sim_require_nnan=True,
            nc=nc,
        )
        return tuple(outs)

    devices = jax.devices()[:NCORE]
    mesh = Mesh(np.asarray(devices), ("core",))
    fn = jax.jit(
        shard_map(_body, mesh=mesh,
                  in_specs=(PartitionSpec("core"),) * (n_params + n_outs),
                  out_specs=(PartitionSpec("core"),) * n_outs,
                  check_rep=False),
        donate_argnums=donate, keep_unused=True,
    )
    sharding = NamedSharding(mesh, PartitionSpec("core"))
    return fn, in_names, out_names, out_avals, zero_shapes, sharding


def _get_fast(nc):
    if "fast" not in _CACHE:
        _CACHE["fast"] = _prep_fast(nc)
    return _CACHE["fast"]


def kernel(mean, logvar, x):
    import jax
    assert mean.shape == (B, C, H, W), mean.shape
    nc = _get_nc()
    fn, in_names, out_names, out_avals, zero_shapes, sharding = _get_fast(nc)
    full = {
        "m_in": np.asarray(mean, dtype=np.float32).reshape(NCORE * P, FREE).astype(NPBF),
        "lv_in": np.asarray(logvar, dtype=np.float32).reshape(NCORE * P, FREE).astype(NPBF),
        "x_in": np.asarray(x, dtype=np.float32).reshape(NCORE * P, FREE).astype(NPBF),
        "c_in": np.tile(_consts_np(), (NCORE, 1)),
    }
    args = [jax.device_put(full[nm], sharding) for nm in in_names]
    zeros = [jax.device_put(np.zeros((NCORE * s[0], *s[1:]), d), sharding)
             for (s, d) in zero_shapes]
    outs = fn(*args, *zeros)
    oi = out_names.index("o_out")
    res = np.asarray(outs[oi]).reshape(NCORE, SPB)
    return res.reshape(B).astype(np.float32)


if __name__ == "__main__":
    rng = np.random.default_rng(0)
    m = (rng.standard_normal((B, C, H, W)) * 0.1).astype(np.float32)
    lv = (rng.standard_normal((B, C, H, W)) * 0.1 - 2.0).astype(np.float32)
    xx = rng.uniform(-1.0, 1.0 - 1e-6, (B, C, H, W)).astype(np.float32)
    out = kernel(m, lv, xx)
    print("kernel out[:8]:", out[:8])
